# revision 20
# baseline (speedup 1.0000x reference)
"""CrossAttention Trainium2 kernel (nn_CrossAttention_28544352649420).

Full-input contract: kernel(**inputs) takes the unsharded arrays
  inputA [8,2048,1024] f32, inputB [8,2048,1024] f32,
  maskA [8,2048] f32, maskB [8,2048] f32, W [1024,1024] f32, b [1024] f32
and returns (cvA [8,2048,1024], cvB [8,2048,1024]) matching

  projA  = inputA @ W + b
  scores = projA @ inputB^T, masked_fill(maskA x maskB == 0, -1e9)
  attnA  = softmax(scores, axis=1); attnB = softmax(scores, axis=2)
  cvA    = attnA^T @ inputA;        cvB = attnB @ inputB

Sharding: batch dim across the 8 NeuronCores (data parallel, SPMD —
one batch element per core; every core holds the full W).

Per-core schedule (B=1, La=Lb=2048, Da=Db=1024), fp32r matmuls
(fp32-ish operands at 1 cycle/row for free-size >= 256):
  Preamble: WT = W^T (PE transpose), then per m-pair (256 cols)
            transpose B strips and accumulate K = W @ B^T (f32r) into
            SBUF ([d,m], 64KB/part), plus bv = b @ B^T (rank-1 bias
            row; scores = A @ K + 1*bv) and b_bf (bf16 B copy).
  Pass 1 (16 l-strips, 3-stage software pipeline):
    stage_s: transpose the A strip (PE), S strip = 1*bv + A @ K via
             f32r matmuls (4 psum buffers), maskB min into smask (DVE).
    stage_t: rowmax (DVE), E_B = exp(S - rowmax) bf16 + fused denom
             (ACT), maskA min (gpsimd) -> TMIN DRAM scratch, running
             colmax via gpsimd partition_all_reduce + max merge.
    stage_v: PE transpose of E_B -> cvB strip = E_B^T @ inputB_bf/denom.
  Phase C (16 m-chunks, software-pipelined, 3 ahead): load the
    masked-score column slab, E_A = exp(s - colmax) bf16 (sub on
    gpsimd, exp on ACT), cvA chunk = E_A^T @ inputA_bf / colsum
    (ones-vector matmul colsum).
"""
import sys

sys.path.insert(0, "/opt/trn_rl_repo")

import numpy as np
from contextlib import ExitStack

import concourse.bass as bass
import concourse.tile as tile
from concourse import bacc
from concourse import mybir
from concourse import bass_isa
from concourse.bass_utils import run_bass_kernel_spmd
from concourse.masks import make_identity

F32 = mybir.dt.float32
F32R = mybir.dt.float32r
BF16 = mybir.dt.bfloat16
MIN = mybir.AluOpType.min
MULT = mybir.AluOpType.mult
ADD = mybir.AluOpType.add
SUB = mybir.AluOpType.subtract
MAXOP = mybir.AluOpType.max
EXP = mybir.ActivationFunctionType.Exp
X = mybir.AxisListType.X

B, L, D = 8, 2048, 1024
NS = L // 128  # 16 strips
KC = D // 128  # 8 contraction chunks
BIG = 1.0e30
NEG = -1.0e9

_CACHE = {}


def build():
    nc = bacc.Bacc(trn_type="TRN2")

    inputA = nc.declare_dram_parameter("inputA", [L, D], F32, isOutput=False)
    inputB = nc.declare_dram_parameter("inputB", [L, D], F32, isOutput=False)
    maskA = nc.declare_dram_parameter("maskA", [L, 1], F32, isOutput=False)
    maskB = nc.declare_dram_parameter("maskB", [1, L], F32, isOutput=False)
    Wp = nc.declare_dram_parameter("W", [D, D], F32, isOutput=False)
    bp = nc.declare_dram_parameter("b", [D, 1], F32, isOutput=False)
    cvA = nc.declare_dram_parameter("cvA", [L, D], F32, isOutput=True)
    cvB = nc.declare_dram_parameter("cvB", [L, D], F32, isOutput=True)

    TMIN_d = nc.dram_tensor("TMIN_d", [L, L], F32)  # fully-masked scores

    def r(ap):
        return ap.bitcast(F32R)

    with tile.TileContext(nc) as tc, ExitStack() as ctx:
        glob = ctx.enter_context(tc.tile_pool(name="glob", bufs=1))

        # identities first: the first PE transposes depend only on these
        ident_bf = glob.tile([128, 128], BF16)
        make_identity(nc, ident_bf)
        ident_f = glob.tile([128, 128], F32)
        make_identity(nc, ident_f)
        ident_r = glob.tile([128, 128], F32R)
        nc.vector.tensor_copy(out=ident_r, in_=ident_f)

        ones_bf = glob.tile([128, 1], BF16)
        nc.vector.memset(ones_bf, 1.0)
        ones_f = glob.tile([1, 128], F32)
        nc.vector.memset(ones_f, 1.0)
        ones_r = glob.tile([1, 128], F32R)  # rank-1 bias lhsT
        nc.vector.tensor_copy(out=ones_r, in_=ones_f)

        b_t = glob.tile([128, KC], F32)
        b_tr = glob.tile([128, KC], F32R)
        maA = glob.tile([128, NS], F32)
        maA_min = glob.tile([128, NS], F32)  # 1 -> +BIG, 0 -> NEG
        MBb = glob.tile([128, L], BF16)  # maskB min-mask
        bv = glob.tile([1, L], F32R)  # b @ B^T (rank-1 score bias)
        cmax_full = glob.tile([128, L], F32)  # running colmax of masked scores
        nc.gpsimd.memset(cmax_full, -3.0e38)
        a_bf = glob.tile([128, NS, D], BF16)  # [l-part, lc, e] for phase C
        b_bf = glob.tile([128, NS, D], BF16)  # [m-part, mc, e] for cvB

        # K-pool spans preamble + pass 1; freed before phase C
        with tc.tile_pool(name="kpool", bufs=1) as kpl:
            K = kpl.tile([128, KC, L], F32R)  # K[p,dc,m] = (W @ B^T)[dc*128+p, m]

            # small-globals scope (temp f32 maskB row freed before main preamble)
            with tc.tile_pool(name="pre0", bufs=1) as pre0:
                # small-input DMAs go on the ACT hwdge queue so the W/B strip
                # loads on the SP queue are not stuck behind them
                nc.scalar.dma_start(
                    out=b_t, in_=bp[:].rearrange("(c p) o -> p (c o)", p=128)
                )
                nc.scalar.dma_start(
                    out=maA, in_=maskA[:].rearrange("(s p) o -> p (s o)", p=128)
                )
                nc.vector.tensor_scalar(
                    out=maA_min, in0=maA, scalar1=BIG - NEG, scalar2=NEG,
                    op0=MULT, op1=ADD,
                )
                nc.vector.tensor_copy(out=b_tr, in_=b_t)
                MBf = pre0.tile([128, L], F32)
                nc.scalar.dma_start(
                    out=MBf,
                    in_=maskB[:].rearrange("o n -> (o n)").partition_broadcast(128),
                )
                nc.vector.tensor_scalar(
                    out=MBb, in0=MBf, scalar1=BIG - NEG, scalar2=NEG,
                    op0=MULT, op1=ADD,
                )

            # ---------------- Preamble: WT, K = W @ B^T, bv, b_bf ----------------
            with tc.tile_pool(name="pre", bufs=1) as pre, \
                 tc.tile_pool(name="pre_ps", bufs=1, space="PSUM") as pre_ps:
                WT = pre.tile([128, KC, D], F32R)  # WT[p,ec,d] = W[d, ec*128+p]
                for wc in range(KC):
                    stripW = pre.tile([128, D], F32R, tag="strip", bufs=2)
                    nc.scalar.dma_start(
                        out=stripW, in_=r(Wp[wc * 128:(wc + 1) * 128, :])
                    )
                    for g in range(2):
                        tpw = pre_ps.tile([128, 4, 128], F32R, tag="tp", bufs=2)
                        for j in range(4):
                            ec = g * 4 + j
                            nc.tensor.transpose(
                                tpw[:, j, :],
                                stripW[:, ec * 128:(ec + 1) * 128],
                                ident_r,
                            )
                        nc.scalar.copy(
                            out=WT[:, g * 4:(g + 1) * 4, wc * 128:(wc + 1) * 128],
                            in_=tpw,
                        )

                for p in range(8):  # m-pairs of 256
                    btcol = pre.tile([128, KC, 256], F32R, tag="btcol", bufs=2)
                    for s in range(2):
                        mc = p * 2 + s
                        stripB = pre.tile([128, D], F32R, tag="strip", bufs=2)
                        nc.sync.dma_start(
                            out=stripB, in_=r(inputB[mc * 128:(mc + 1) * 128, :])
                        )
                        nc.gpsimd.tensor_copy(out=b_bf[:, mc, :], in_=stripB)
                        for g in range(2):
                            tpb = pre_ps.tile([128, 4, 128], F32R, tag="tp", bufs=2)
                            for j in range(4):
                                ec = g * 4 + j
                                nc.tensor.transpose(
                                    tpb[:, j, :],
                                    stripB[:, ec * 128:(ec + 1) * 128],
                                    ident_r,
                                )
                            nc.scalar.copy(
                                out=btcol[:, g * 4:(g + 1) * 4,
                                          s * 128:(s + 1) * 128],
                                in_=tpb,
                            )
                    # bias row bv[p-slice] = b @ B^T
                    bvp = pre_ps.tile([1, 256], F32, tag="bvp", bufs=1)
                    for ec in range(KC):
                        nc.tensor.matmul(
                            bvp,
                            b_tr[:, ec:ec + 1],
                            btcol[:, ec, :],
                            start=(ec == 0),
                            stop=(ec == KC - 1),
                        )
                    nc.scalar.copy(out=bv[0:1, p * 256:(p + 1) * 256], in_=bvp)
                    # K[:, :, p-slice] accumulation (f32r, ap=256)
                    for h in range(4):  # dc pairs
                        kps = pre_ps.tile([128, 2, 256], F32, tag="kps", bufs=2)
                        for dd in range(2):
                            dc = h * 2 + dd
                            for ec in range(KC):
                                nc.tensor.matmul(
                                    kps[:, dd, :],
                                    WT[:, ec, dc * 128:(dc + 1) * 128],
                                    btcol[:, ec, :],
                                    start=(ec == 0),
                                    stop=(ec == KC - 1),
                                )
                        nc.scalar.copy(
                            out=K[:, h * 2:(h + 1) * 2, p * 256:(p + 1) * 256],
                            in_=kps,
                        )

            # ---------------- Pass 1: S strips, E_B, cvB, colmax ----------------
            with tc.tile_pool(name="p1", bufs=1) as p1, \
                 tc.tile_pool(name="p1_ps", bufs=1, space="PSUM") as p1_ps:
                smasks = {}
                ebs = {}
                astrips = {}

                def a_load(i):
                    t = p1.tile([128, D], F32R, tag="stripA", bufs=2)
                    nc.sync.dma_start(
                        out=t, in_=r(inputA[i * 128:(i + 1) * 128, :])
                    )
                    astrips[i] = t

                a_load(0)

                ats = {}

                def stage_s_pre(i):
                    # A strip (prefetched): PE transpose + at copy (ACT first)
                    stripA = astrips.pop(i)
                    if i + 1 < NS:
                        a_load(i + 1)
                    tpa = p1_ps.tile([128, KC, 128], F32R, tag="tpa", bufs=1)
                    for dc in range(KC):
                        nc.tensor.transpose(
                            tpa[:, dc, :],
                            stripA[:, dc * 128:(dc + 1) * 128],
                            ident_r,
                        )
                    at = p1.tile([128, KC, 128], F32R, tag="at", bufs=2)
                    nc.scalar.copy(out=at, in_=tpa)
                    nc.gpsimd.tensor_copy(out=a_bf[:, i, :], in_=stripA)
                    ats[i] = at

                def stage_s_mm(i):
                    # S matmuls + maskB min
                    at = ats.pop(i)
                    smask = p1.tile([128, L], F32, tag="smask", bufs=2)
                    for q in range(4):
                        sps = p1_ps.tile([128, 512], F32, tag="ps2k", bufs=4)
                        msl = slice(q * 512, (q + 1) * 512)
                        nc.tensor.matmul(
                            sps, ones_r, bv[0:1, msl], start=True, stop=False,
                        )
                        for dc in range(KC):
                            nc.tensor.matmul(
                                sps,
                                at[:, dc, :],
                                K[:, dc, msl],
                                start=False,
                                stop=(dc == KC - 1),
                            )
                        nc.vector.tensor_tensor(
                            out=smask[:, msl], in0=sps, in1=MBb[:, msl], op=MIN
                        )
                    smasks[i] = smask

                def stage_t(i):
                    # row softmax stats, E_B, TMIN scratch, running colmax
                    smask = smasks.pop(i)
                    negrm = p1.tile([128, 1], F32, tag="negrm", bufs=2)
                    nc.vector.reduce_max(out=negrm, in_=smask, axis=X, negate=True)
                    biasB = p1.tile([128, 1], F32, tag="biasB", bufs=2)
                    nc.vector.tensor_tensor(
                        out=biasB, in0=negrm, in1=maA[:, i:i + 1], op=MULT
                    )
                    eb = p1.tile([128, L], BF16, tag="eb", bufs=2)
                    denomB = p1.tile([128, 1], F32, tag="denomB", bufs=2)
                    nc.scalar.activation(
                        out=eb, in_=smask, func=EXP,
                        bias=biasB, scale=maA[:, i:i + 1], accum_out=denomB,
                    )
                    # fully-masked scores (A-mask applied too) -> DRAM for phase C
                    if i == NS - 1:
                        nc.vector.tensor_scalar_min(smask, smask, maA_min[:, i:i + 1])
                    else:
                        nc.gpsimd.tensor_scalar_min(smask, smask, maA_min[:, i:i + 1])
                    nc.sync.dma_start(out=TMIN_d[i * 128:(i + 1) * 128, :], in_=smask)
                    # per-strip column max -> running colmax (gpsimd engine)
                    for h in range(2):
                        ar = p1.tile([128, 1024], F32, tag="ar", bufs=1)
                        hsl = slice(h * 1024, (h + 1) * 1024)
                        nc.gpsimd.partition_all_reduce(
                            ar, smask[:, hsl], channels=128,
                            reduce_op=bass_isa.ReduceOp.max,
                        )
                        nc.vector.tensor_tensor(
                            out=cmax_full[:, hsl], in0=cmax_full[:, hsl],
                            in1=ar, op=MAXOP,
                        )
                    ebs[i] = (eb, denomB)

                def stage_v(i):
                    # cvB strip
                    eb, denomB = ebs.pop(i)
                    ebt = p1.tile([128, NS, 128], BF16, tag="ebt", bufs=2)
                    for g in range(2):
                        tp3 = p1_ps.tile([128, 8, 128], BF16, tag="ps2k", bufs=4)
                        for j in range(8):
                            mc = g * 8 + j
                            nc.tensor.transpose(
                                tp3[:, j, :], eb[:, mc * 128:(mc + 1) * 128],
                                ident_bf,
                            )
                        nc.scalar.copy(out=ebt[:, g * 8:(g + 1) * 8, :], in_=tp3)
                    ups = p1_ps.tile([128, D], F32, tag="ups", bufs=1)
                    for nb in range(2):
                        for mc in range(NS):
                            nc.tensor.matmul(
                                ups[:, nb * 512:(nb + 1) * 512],
                                ebt[:, mc, :],
                                b_bf[:, mc, nb * 512:(nb + 1) * 512],
                                start=(mc == 0),
                                stop=(mc == NS - 1),
                            )
                    rden = p1.tile([128, 1], F32, tag="rden", bufs=2)
                    nc.vector.reciprocal(out=rden, in_=denomB)
                    cvb_sb = p1.tile([128, D], F32, tag="cvb_sb", bufs=1)
                    nc.vector.tensor_scalar(
                        out=cvb_sb, in0=ups, scalar1=rden, scalar2=None, op0=MULT
                    )
                    nc.sync.dma_start(out=cvB[i * 128:(i + 1) * 128, :], in_=cvb_sb)

                for i in range(NS):
                    stage_s_pre(i)
                    if i >= 2:
                        stage_v(i - 2)
                    stage_s_mm(i)
                    if i >= 1:
                        stage_t(i - 1)
                stage_t(NS - 1)
                stage_v(NS - 2)
                stage_v(NS - 1)

        # ---------------- Phase C: cvA per m-chunk ----------------
        with tc.tile_pool(name="pc", bufs=1) as pc, \
             tc.tile_pool(name="pc_ps", bufs=1, space="PSUM") as pc_ps:
            cprev = {}
            subs_last = {}

            def c_stage1(j):
                # tmin column slab: [l-part, lc, m] for 128 columns m
                slab = pc.tile([128, NS, 128], F32, tag="slab", bufs=6)
                if j < 3:
                    nc.sync.dma_start(
                        out=slab[:, 0:NS - 1, :],
                        in_=TMIN_d[0:(NS - 1) * 128, j * 128:(j + 1) * 128]
                        .rearrange("(c p) m -> p c m", p=128),
                    )
                    nc.sync.dma_start(
                        out=slab[:, NS - 1, :],
                        in_=TMIN_d[(NS - 1) * 128:, j * 128:(j + 1) * 128]
                        .rearrange("(c p) m -> p (c m)", p=128),
                    )
                else:
                    nc.sync.dma_start(
                        out=slab,
                        in_=TMIN_d[:, j * 128:(j + 1) * 128].rearrange(
                            "(c p) m -> p c m", p=128
                        ),
                    )
                # E_A tiles = exp(tmin - colmax), bf16 (colmax from pass 1)
                cmb = cmax_full[:, j * 128:(j + 1) * 128].unsqueeze(1).broadcast_to(
                    (128, 4, 128)
                )
                eng = nc.vector if j < 3 else nc.gpsimd
                for q in range(4):
                    sub_inst = eng.tensor_tensor(
                        out=slab[:, q * 4:(q + 1) * 4, :],
                        in0=slab[:, q * 4:(q + 1) * 4, :], in1=cmb, op=SUB,
                    )
                subs_last[j] = sub_inst
                ea_t = pc.tile([128, NS, 128], BF16, tag="ea_t", bufs=6)
                for hh in range(4):
                    nc.scalar.activation(
                        out=ea_t[:, hh * 4:(hh + 1) * 4, :],
                        in_=slab[:, hh * 4:(hh + 1) * 4, :], func=EXP,
                    )
                cprev[j] = ea_t

            def c_stage2(j):
                ea_t = cprev.pop(j)
                aps = pc_ps.tile([128, D], F32, tag="aps", bufs=3)
                csum = pc_ps.tile([128, 1], F32, tag="csum", bufs=2)
                for lc in range(NS):
                    nc.tensor.matmul(
                        csum, ea_t[:, lc, :], ones_bf,
                        start=(lc == 0), stop=(lc == NS - 1),
                    )
                for nb in range(2):
                    for lc in range(NS):
                        nc.tensor.matmul(
                            aps[:, nb * 512:(nb + 1) * 512],
                            ea_t[:, lc, :],
                            a_bf[:, lc, nb * 512:(nb + 1) * 512],
                            start=(lc == 0),
                            stop=(lc == NS - 1),
                        )
                rcs = pc.tile([128, 1], F32, tag="rcs", bufs=2)
                nc.vector.reciprocal(out=rcs, in_=csum)
                cva_sb = pc.tile([128, D], F32, tag="cva_sb", bufs=2)
                nc.vector.tensor_scalar(
                    out=cva_sb, in0=aps, scalar1=rcs, scalar2=None, op0=MULT
                )
                nc.sync.dma_start(out=cvA[j * 128:(j + 1) * 128, :], in_=cva_sb)

            c_stage1(0)
            c_stage1(1)
            c_stage1(2)
            c_stage1(3)
            for j in range(NS):
                if j + 4 < NS:
                    c_stage1(j + 4)
                c_stage2(j)
    if not nc.is_finalized():
        nc.finalize()
    return nc


def run(inputs, trace=False, trace_kwargs=None):
    if "nc" not in _CACHE:
        _CACHE["nc"] = build()
    nc = _CACHE["nc"]
    in_maps = []
    for i in range(B):
        in_maps.append({
            "inputA": np.ascontiguousarray(inputs["inputA"][i], dtype=np.float32),
            "inputB": np.ascontiguousarray(inputs["inputB"][i], dtype=np.float32),
            "maskA": np.ascontiguousarray(
                inputs["maskA"][i], dtype=np.float32).reshape(L, 1),
            "maskB": np.ascontiguousarray(
                inputs["maskB"][i], dtype=np.float32).reshape(1, L),
            "W": np.ascontiguousarray(inputs["W"], dtype=np.float32),
            "b": np.ascontiguousarray(inputs["b"], dtype=np.float32).reshape(D, 1),
        })
    try:
        res = run_bass_kernel_spmd(
            nc, in_maps, core_ids=list(range(B)), trace=trace,
            **(trace_kwargs or {}),
        )
    except ModuleNotFoundError:
        res = run_bass_kernel_spmd(nc, in_maps, core_ids=list(range(B)), trace=False)
    cva = np.stack([res.results[i]["cvA"] for i in range(B)]).astype(np.float32)
    cvb = np.stack([res.results[i]["cvB"] for i in range(B)]).astype(np.float32)
    return (cva, cvb), res


def kernel(**inputs):
    (cva, cvb), _ = run(inputs, trace=False)
    return cva, cvb


# revision 21
# speedup vs baseline: 1.0026x; 1.0026x over previous
"""CrossAttention Trainium2 kernel (nn_CrossAttention_28544352649420).

Full-input contract: kernel(**inputs) takes the unsharded arrays
  inputA [8,2048,1024] f32, inputB [8,2048,1024] f32,
  maskA [8,2048] f32, maskB [8,2048] f32, W [1024,1024] f32, b [1024] f32
and returns (cvA [8,2048,1024], cvB [8,2048,1024]) matching

  projA  = inputA @ W + b
  scores = projA @ inputB^T, masked_fill(maskA x maskB == 0, -1e9)
  attnA  = softmax(scores, axis=1); attnB = softmax(scores, axis=2)
  cvA    = attnA^T @ inputA;        cvB = attnB @ inputB

Sharding: batch dim across the 8 NeuronCores (data parallel, SPMD —
one batch element per core; every core holds the full W).

Per-core schedule (B=1, La=Lb=2048, Da=Db=1024), fp32r matmuls
(fp32-ish operands at 1 cycle/row for free-size >= 256):
  Preamble: WT = W^T (PE transpose), then per m-pair (256 cols)
            transpose B strips and accumulate K = W @ B^T (f32r) into
            SBUF ([d,m], 64KB/part), plus bv = b @ B^T (rank-1 bias
            row; scores = A @ K + 1*bv) and b_bf (bf16 B copy).
  Pass 1 (16 l-strips, 3-stage software pipeline):
    stage_s: transpose the A strip (PE), S strip = 1*bv + A @ K via
             f32r matmuls (4 psum buffers), maskB min into smask (DVE).
    stage_t: rowmax (DVE), E_B = exp(S - rowmax) bf16 + fused denom
             (ACT), maskA min (gpsimd) -> TMIN DRAM scratch, running
             colmax via gpsimd partition_all_reduce + max merge.
    stage_v: PE transpose of E_B -> cvB strip = E_B^T @ inputB_bf/denom.
  Phase C (16 m-chunks, software-pipelined, 3 ahead): load the
    masked-score column slab, E_A = exp(s - colmax) bf16 (sub on
    gpsimd, exp on ACT), cvA chunk = E_A^T @ inputA_bf / colsum
    (ones-vector matmul colsum).
"""
import sys

sys.path.insert(0, "/opt/trn_rl_repo")

import numpy as np
from contextlib import ExitStack

import concourse.bass as bass
import concourse.tile as tile
from concourse import bacc
from concourse import mybir
from concourse import bass_isa
from concourse.bass_utils import run_bass_kernel_spmd
from concourse.masks import make_identity

F32 = mybir.dt.float32
F32R = mybir.dt.float32r
BF16 = mybir.dt.bfloat16
MIN = mybir.AluOpType.min
MULT = mybir.AluOpType.mult
ADD = mybir.AluOpType.add
SUB = mybir.AluOpType.subtract
MAXOP = mybir.AluOpType.max
EXP = mybir.ActivationFunctionType.Exp
X = mybir.AxisListType.X

B, L, D = 8, 2048, 1024
NS = L // 128  # 16 strips
KC = D // 128  # 8 contraction chunks
BIG = 1.0e30
NEG = -1.0e9

_CACHE = {}


def build():
    nc = bacc.Bacc(trn_type="TRN2")

    inputA = nc.declare_dram_parameter("inputA", [L, D], F32, isOutput=False)
    inputB = nc.declare_dram_parameter("inputB", [L, D], F32, isOutput=False)
    maskA = nc.declare_dram_parameter("maskA", [L, 1], F32, isOutput=False)
    maskB = nc.declare_dram_parameter("maskB", [1, L], F32, isOutput=False)
    Wp = nc.declare_dram_parameter("W", [D, D], F32, isOutput=False)
    bp = nc.declare_dram_parameter("b", [D, 1], F32, isOutput=False)
    cvA = nc.declare_dram_parameter("cvA", [L, D], F32, isOutput=True)
    cvB = nc.declare_dram_parameter("cvB", [L, D], F32, isOutput=True)

    TMIN_d = nc.dram_tensor("TMIN_d", [L, L], F32)  # fully-masked scores

    def r(ap):
        return ap.bitcast(F32R)

    with tile.TileContext(nc) as tc, ExitStack() as ctx:
        glob = ctx.enter_context(tc.tile_pool(name="glob", bufs=1))

        # identities first: the first PE transposes depend only on these
        ident_bf = glob.tile([128, 128], BF16)
        make_identity(nc, ident_bf)
        ident_f = glob.tile([128, 128], F32)
        make_identity(nc, ident_f)
        ident_r = glob.tile([128, 128], F32R)
        nc.vector.tensor_copy(out=ident_r, in_=ident_f)

        ones_bf = glob.tile([128, 1], BF16)
        nc.vector.memset(ones_bf, 1.0)
        ones_f = glob.tile([1, 128], F32)
        nc.vector.memset(ones_f, 1.0)
        ones_r = glob.tile([1, 128], F32R)  # rank-1 bias lhsT
        nc.vector.tensor_copy(out=ones_r, in_=ones_f)

        b_t = glob.tile([128, KC], F32)
        b_tr = glob.tile([128, KC], F32R)
        maA = glob.tile([128, NS], F32)
        maA_min = glob.tile([128, NS], F32)  # 1 -> +BIG, 0 -> NEG
        MBb = glob.tile([128, L], BF16)  # maskB min-mask
        bv = glob.tile([1, L], F32R)  # b @ B^T (rank-1 score bias)
        cmax_full = glob.tile([128, L], F32)  # running colmax of masked scores
        nc.gpsimd.memset(cmax_full, -3.0e38)
        a_bf = glob.tile([128, NS, D], BF16)  # [l-part, lc, e] for phase C
        b_bf = glob.tile([128, NS, D], BF16)  # [m-part, mc, e] for cvB

        # K-pool spans preamble + pass 1; freed before phase C
        with tc.tile_pool(name="kpool", bufs=1) as kpl:
            K = kpl.tile([128, KC, L], F32R)  # K[p,dc,m] = (W @ B^T)[dc*128+p, m]

            # small-globals scope (temp f32 maskB row freed before main preamble)
            with tc.tile_pool(name="pre0", bufs=1) as pre0:
                # small-input DMAs go on the ACT hwdge queue so the W/B strip
                # loads on the SP queue are not stuck behind them
                nc.scalar.dma_start(
                    out=b_t, in_=bp[:].rearrange("(c p) o -> p (c o)", p=128)
                )
                nc.scalar.dma_start(
                    out=maA, in_=maskA[:].rearrange("(s p) o -> p (s o)", p=128)
                )
                nc.vector.tensor_scalar(
                    out=maA_min, in0=maA, scalar1=BIG - NEG, scalar2=NEG,
                    op0=MULT, op1=ADD,
                )
                nc.vector.tensor_copy(out=b_tr, in_=b_t)
                MBf = pre0.tile([128, L], F32)
                nc.scalar.dma_start(
                    out=MBf,
                    in_=maskB[:].rearrange("o n -> (o n)").partition_broadcast(128),
                )
                nc.vector.tensor_scalar(
                    out=MBb, in0=MBf, scalar1=BIG - NEG, scalar2=NEG,
                    op0=MULT, op1=ADD,
                )

            # ---------------- Preamble: WT, K = W @ B^T, bv, b_bf ----------------
            with tc.tile_pool(name="pre", bufs=1) as pre, \
                 tc.tile_pool(name="pre_ps", bufs=1, space="PSUM") as pre_ps:
                WT = pre.tile([128, KC, D], F32R)  # WT[p,ec,d] = W[d, ec*128+p]
                for wc in range(KC):
                    stripW = pre.tile([128, D], F32R, tag="strip", bufs=2)
                    nc.scalar.dma_start(
                        out=stripW, in_=r(Wp[wc * 128:(wc + 1) * 128, :])
                    )
                    for g in range(2):
                        tpw = pre_ps.tile([128, 4, 128], F32R, tag="tp", bufs=2)
                        for j in range(4):
                            ec = g * 4 + j
                            nc.tensor.transpose(
                                tpw[:, j, :],
                                stripW[:, ec * 128:(ec + 1) * 128],
                                ident_r,
                            )
                        nc.scalar.copy(
                            out=WT[:, g * 4:(g + 1) * 4, wc * 128:(wc + 1) * 128],
                            in_=tpw,
                        )

                for p in range(8):  # m-pairs of 256
                    btcol = pre.tile([128, KC, 256], F32R, tag="btcol", bufs=2)
                    for s in range(2):
                        mc = p * 2 + s
                        stripB = pre.tile([128, D], F32R, tag="strip", bufs=2)
                        nc.sync.dma_start(
                            out=stripB, in_=r(inputB[mc * 128:(mc + 1) * 128, :])
                        )
                        nc.gpsimd.tensor_copy(out=b_bf[:, mc, :], in_=stripB)
                        for g in range(2):
                            tpb = pre_ps.tile([128, 4, 128], F32R, tag="tp", bufs=2)
                            for j in range(4):
                                ec = g * 4 + j
                                nc.tensor.transpose(
                                    tpb[:, j, :],
                                    stripB[:, ec * 128:(ec + 1) * 128],
                                    ident_r,
                                )
                            nc.scalar.copy(
                                out=btcol[:, g * 4:(g + 1) * 4,
                                          s * 128:(s + 1) * 128],
                                in_=tpb,
                            )
                    # K[:, :, p-slice] accumulation (f32r, ap=256)
                    for h in range(4):  # dc pairs
                        kps = pre_ps.tile([128, 2, 256], F32, tag="kps", bufs=2)
                        for dd in range(2):
                            dc = h * 2 + dd
                            for ec in range(KC):
                                nc.tensor.matmul(
                                    kps[:, dd, :],
                                    WT[:, ec, dc * 128:(dc + 1) * 128],
                                    btcol[:, ec, :],
                                    start=(ec == 0),
                                    stop=(ec == KC - 1),
                                )
                        nc.scalar.copy(
                            out=K[:, h * 2:(h + 1) * 2, p * 256:(p + 1) * 256],
                            in_=kps,
                        )
                    # bias row bv[p-slice] = b @ B^T (after K: off the critical path)
                    bvp = pre_ps.tile([1, 256], F32, tag="bvp", bufs=1)
                    for ec in range(KC):
                        nc.tensor.matmul(
                            bvp,
                            b_tr[:, ec:ec + 1],
                            btcol[:, ec, :],
                            start=(ec == 0),
                            stop=(ec == KC - 1),
                        )
                    nc.scalar.copy(out=bv[0:1, p * 256:(p + 1) * 256], in_=bvp)

            # ---------------- Pass 1: S strips, E_B, cvB, colmax ----------------
            with tc.tile_pool(name="p1", bufs=1) as p1, \
                 tc.tile_pool(name="p1_ps", bufs=1, space="PSUM") as p1_ps:
                smasks = {}
                ebs = {}
                astrips = {}

                def a_load(i):
                    t = p1.tile([128, D], F32R, tag="stripA", bufs=2)
                    nc.sync.dma_start(
                        out=t, in_=r(inputA[i * 128:(i + 1) * 128, :])
                    )
                    astrips[i] = t

                a_load(0)

                ats = {}

                def stage_s_pre(i):
                    # A strip (prefetched): PE transpose + at copy (ACT first)
                    stripA = astrips.pop(i)
                    if i + 1 < NS:
                        a_load(i + 1)
                    tpa = p1_ps.tile([128, KC, 128], F32R, tag="tpa", bufs=1)
                    for dc in range(KC):
                        nc.tensor.transpose(
                            tpa[:, dc, :],
                            stripA[:, dc * 128:(dc + 1) * 128],
                            ident_r,
                        )
                    at = p1.tile([128, KC, 128], F32R, tag="at", bufs=2)
                    nc.scalar.copy(out=at, in_=tpa)
                    nc.gpsimd.tensor_copy(out=a_bf[:, i, :], in_=stripA)
                    ats[i] = at

                def stage_s_mm(i):
                    # S matmuls + maskB min
                    at = ats.pop(i)
                    smask = p1.tile([128, L], F32, tag="smask", bufs=2)
                    for q in range(4):
                        sps = p1_ps.tile([128, 512], F32, tag="ps2k", bufs=4)
                        msl = slice(q * 512, (q + 1) * 512)
                        nc.tensor.matmul(
                            sps, ones_r, bv[0:1, msl], start=True, stop=False,
                        )
                        for dc in range(KC):
                            nc.tensor.matmul(
                                sps,
                                at[:, dc, :],
                                K[:, dc, msl],
                                start=False,
                                stop=(dc == KC - 1),
                            )
                        nc.vector.tensor_tensor(
                            out=smask[:, msl], in0=sps, in1=MBb[:, msl], op=MIN
                        )
                    smasks[i] = smask

                def stage_t(i):
                    # row softmax stats, E_B, TMIN scratch, running colmax
                    smask = smasks.pop(i)
                    negrm = p1.tile([128, 1], F32, tag="negrm", bufs=2)
                    nc.vector.reduce_max(out=negrm, in_=smask, axis=X, negate=True)
                    biasB = p1.tile([128, 1], F32, tag="biasB", bufs=2)
                    nc.vector.tensor_tensor(
                        out=biasB, in0=negrm, in1=maA[:, i:i + 1], op=MULT
                    )
                    eb = p1.tile([128, L], BF16, tag="eb", bufs=2)
                    denomB = p1.tile([128, 1], F32, tag="denomB", bufs=2)
                    nc.scalar.activation(
                        out=eb, in_=smask, func=EXP,
                        bias=biasB, scale=maA[:, i:i + 1], accum_out=denomB,
                    )
                    # fully-masked scores (A-mask applied too) -> DRAM for phase C
                    if i == NS - 1:
                        nc.vector.tensor_scalar_min(smask, smask, maA_min[:, i:i + 1])
                    else:
                        nc.gpsimd.tensor_scalar_min(smask, smask, maA_min[:, i:i + 1])
                    nc.sync.dma_start(out=TMIN_d[i * 128:(i + 1) * 128, :], in_=smask)
                    # per-strip column max -> running colmax (gpsimd engine)
                    for h in range(2):
                        ar = p1.tile([128, 1024], F32, tag="ar", bufs=1)
                        hsl = slice(h * 1024, (h + 1) * 1024)
                        nc.gpsimd.partition_all_reduce(
                            ar, smask[:, hsl], channels=128,
                            reduce_op=bass_isa.ReduceOp.max,
                        )
                        nc.vector.tensor_tensor(
                            out=cmax_full[:, hsl], in0=cmax_full[:, hsl],
                            in1=ar, op=MAXOP,
                        )
                    ebs[i] = (eb, denomB)

                def stage_v(i):
                    # cvB strip
                    eb, denomB = ebs.pop(i)
                    ebt = p1.tile([128, NS, 128], BF16, tag="ebt", bufs=2)
                    for g in range(2):
                        tp3 = p1_ps.tile([128, 8, 128], BF16, tag="ps2k", bufs=4)
                        for j in range(8):
                            mc = g * 8 + j
                            nc.tensor.transpose(
                                tp3[:, j, :], eb[:, mc * 128:(mc + 1) * 128],
                                ident_bf,
                            )
                        nc.scalar.copy(out=ebt[:, g * 8:(g + 1) * 8, :], in_=tp3)
                    ups = p1_ps.tile([128, D], F32, tag="ups", bufs=1)
                    for nb in range(2):
                        for mc in range(NS):
                            nc.tensor.matmul(
                                ups[:, nb * 512:(nb + 1) * 512],
                                ebt[:, mc, :],
                                b_bf[:, mc, nb * 512:(nb + 1) * 512],
                                start=(mc == 0),
                                stop=(mc == NS - 1),
                            )
                    rden = p1.tile([128, 1], F32, tag="rden", bufs=2)
                    nc.vector.reciprocal(out=rden, in_=denomB)
                    cvb_sb = p1.tile([128, D], F32, tag="cvb_sb", bufs=1)
                    nc.vector.tensor_scalar(
                        out=cvb_sb, in0=ups, scalar1=rden, scalar2=None, op0=MULT
                    )
                    nc.sync.dma_start(out=cvB[i * 128:(i + 1) * 128, :], in_=cvb_sb)

                for i in range(NS):
                    stage_s_pre(i)
                    if i >= 2:
                        stage_v(i - 2)
                    stage_s_mm(i)
                    if i >= 1:
                        stage_t(i - 1)
                stage_t(NS - 1)
                stage_v(NS - 2)
                stage_v(NS - 1)

        # ---------------- Phase C: cvA per m-chunk ----------------
        with tc.tile_pool(name="pc", bufs=1) as pc, \
             tc.tile_pool(name="pc_ps", bufs=1, space="PSUM") as pc_ps:
            cprev = {}
            subs_last = {}

            def c_stage1(j):
                # tmin column slab: [l-part, lc, m] for 128 columns m
                slab = pc.tile([128, NS, 128], F32, tag="slab", bufs=6)
                if j < 3:
                    nc.sync.dma_start(
                        out=slab[:, 0:NS - 1, :],
                        in_=TMIN_d[0:(NS - 1) * 128, j * 128:(j + 1) * 128]
                        .rearrange("(c p) m -> p c m", p=128),
                    )
                    nc.sync.dma_start(
                        out=slab[:, NS - 1, :],
                        in_=TMIN_d[(NS - 1) * 128:, j * 128:(j + 1) * 128]
                        .rearrange("(c p) m -> p (c m)", p=128),
                    )
                else:
                    nc.sync.dma_start(
                        out=slab,
                        in_=TMIN_d[:, j * 128:(j + 1) * 128].rearrange(
                            "(c p) m -> p c m", p=128
                        ),
                    )
                # E_A tiles = exp(tmin - colmax), bf16 (colmax from pass 1)
                cmb = cmax_full[:, j * 128:(j + 1) * 128].unsqueeze(1).broadcast_to(
                    (128, 4, 128)
                )
                eng = nc.vector if j < 3 else nc.gpsimd
                for q in range(4):
                    sub_inst = eng.tensor_tensor(
                        out=slab[:, q * 4:(q + 1) * 4, :],
                        in0=slab[:, q * 4:(q + 1) * 4, :], in1=cmb, op=SUB,
                    )
                subs_last[j] = sub_inst
                ea_t = pc.tile([128, NS, 128], BF16, tag="ea_t", bufs=6)
                for hh in range(4):
                    nc.scalar.activation(
                        out=ea_t[:, hh * 4:(hh + 1) * 4, :],
                        in_=slab[:, hh * 4:(hh + 1) * 4, :], func=EXP,
                    )
                cprev[j] = ea_t

            def c_stage2(j):
                ea_t = cprev.pop(j)
                aps = pc_ps.tile([128, D], F32, tag="aps", bufs=3)
                csum = pc_ps.tile([128, 1], F32, tag="csum", bufs=2)
                for lc in range(NS):
                    nc.tensor.matmul(
                        csum, ea_t[:, lc, :], ones_bf,
                        start=(lc == 0), stop=(lc == NS - 1),
                    )
                for nb in range(2):
                    for lc in range(NS):
                        nc.tensor.matmul(
                            aps[:, nb * 512:(nb + 1) * 512],
                            ea_t[:, lc, :],
                            a_bf[:, lc, nb * 512:(nb + 1) * 512],
                            start=(lc == 0),
                            stop=(lc == NS - 1),
                        )
                rcs = pc.tile([128, 1], F32, tag="rcs", bufs=2)
                nc.vector.reciprocal(out=rcs, in_=csum)
                cva_sb = pc.tile([128, D], F32, tag="cva_sb", bufs=2)
                nc.vector.tensor_scalar(
                    out=cva_sb, in0=aps, scalar1=rcs, scalar2=None, op0=MULT
                )
                nc.sync.dma_start(out=cvA[j * 128:(j + 1) * 128, :], in_=cva_sb)

            c_stage1(0)
            c_stage1(1)
            c_stage1(2)
            c_stage1(3)
            for j in range(NS):
                if j + 4 < NS:
                    c_stage1(j + 4)
                c_stage2(j)
    if not nc.is_finalized():
        nc.finalize()
    return nc


def run(inputs, trace=False, trace_kwargs=None):
    if "nc" not in _CACHE:
        _CACHE["nc"] = build()
    nc = _CACHE["nc"]
    in_maps = []
    for i in range(B):
        in_maps.append({
            "inputA": np.ascontiguousarray(inputs["inputA"][i], dtype=np.float32),
            "inputB": np.ascontiguousarray(inputs["inputB"][i], dtype=np.float32),
            "maskA": np.ascontiguousarray(
                inputs["maskA"][i], dtype=np.float32).reshape(L, 1),
            "maskB": np.ascontiguousarray(
                inputs["maskB"][i], dtype=np.float32).reshape(1, L),
            "W": np.ascontiguousarray(inputs["W"], dtype=np.float32),
            "b": np.ascontiguousarray(inputs["b"], dtype=np.float32).reshape(D, 1),
        })
    try:
        res = run_bass_kernel_spmd(
            nc, in_maps, core_ids=list(range(B)), trace=trace,
            **(trace_kwargs or {}),
        )
    except ModuleNotFoundError:
        res = run_bass_kernel_spmd(nc, in_maps, core_ids=list(range(B)), trace=False)
    cva = np.stack([res.results[i]["cvA"] for i in range(B)]).astype(np.float32)
    cvb = np.stack([res.results[i]["cvB"] for i in range(B)]).astype(np.float32)
    return (cva, cvb), res


def kernel(**inputs):
    (cva, cvb), _ = run(inputs, trace=False)
    return cva, cvb


# revision 22
# speedup vs baseline: 1.0035x; 1.0009x over previous
"""CrossAttention Trainium2 kernel (nn_CrossAttention_28544352649420).

Full-input contract: kernel(**inputs) takes the unsharded arrays
  inputA [8,2048,1024] f32, inputB [8,2048,1024] f32,
  maskA [8,2048] f32, maskB [8,2048] f32, W [1024,1024] f32, b [1024] f32
and returns (cvA [8,2048,1024], cvB [8,2048,1024]) matching

  projA  = inputA @ W + b
  scores = projA @ inputB^T, masked_fill(maskA x maskB == 0, -1e9)
  attnA  = softmax(scores, axis=1); attnB = softmax(scores, axis=2)
  cvA    = attnA^T @ inputA;        cvB = attnB @ inputB

Sharding: batch dim across the 8 NeuronCores (data parallel, SPMD —
one batch element per core; every core holds the full W).

Per-core schedule (B=1, La=Lb=2048, Da=Db=1024), fp32r matmuls
(fp32-ish operands at 1 cycle/row for free-size >= 256):
  Preamble: WT = W^T (PE transpose), then per m-pair (256 cols)
            transpose B strips and accumulate K = W @ B^T (f32r) into
            SBUF ([d,m], 64KB/part), plus bv = b @ B^T (rank-1 bias
            row; scores = A @ K + 1*bv) and b_bf (bf16 B copy).
  Pass 1 (16 l-strips, 3-stage software pipeline):
    stage_s: transpose the A strip (PE), S strip = 1*bv + A @ K via
             f32r matmuls (4 psum buffers), maskB min into smask (DVE).
    stage_t: rowmax (DVE), E_B = exp(S - rowmax) bf16 + fused denom
             (ACT), maskA min (gpsimd) -> TMIN DRAM scratch, running
             colmax via gpsimd partition_all_reduce + max merge.
    stage_v: PE transpose of E_B -> cvB strip = E_B^T @ inputB_bf/denom.
  Phase C (16 m-chunks, software-pipelined, 3 ahead): load the
    masked-score column slab, E_A = exp(s - colmax) bf16 (sub on
    gpsimd, exp on ACT), cvA chunk = E_A^T @ inputA_bf / colsum
    (ones-vector matmul colsum).
"""
import sys

sys.path.insert(0, "/opt/trn_rl_repo")

import numpy as np
from contextlib import ExitStack

import concourse.bass as bass
import concourse.tile as tile
from concourse import bacc
from concourse import mybir
from concourse import bass_isa
from concourse.bass_utils import run_bass_kernel_spmd
from concourse.masks import make_identity

F32 = mybir.dt.float32
F32R = mybir.dt.float32r
BF16 = mybir.dt.bfloat16
MIN = mybir.AluOpType.min
MULT = mybir.AluOpType.mult
ADD = mybir.AluOpType.add
SUB = mybir.AluOpType.subtract
MAXOP = mybir.AluOpType.max
EXP = mybir.ActivationFunctionType.Exp
X = mybir.AxisListType.X

B, L, D = 8, 2048, 1024
NS = L // 128  # 16 strips
KC = D // 128  # 8 contraction chunks
BIG = 1.0e30
NEG = -1.0e9

_CACHE = {}


def build():
    nc = bacc.Bacc(trn_type="TRN2")

    inputA = nc.declare_dram_parameter("inputA", [L, D], F32, isOutput=False)
    inputB = nc.declare_dram_parameter("inputB", [L, D], F32, isOutput=False)
    maskA = nc.declare_dram_parameter("maskA", [L, 1], F32, isOutput=False)
    maskB = nc.declare_dram_parameter("maskB", [1, L], F32, isOutput=False)
    Wp = nc.declare_dram_parameter("W", [D, D], F32, isOutput=False)
    bp = nc.declare_dram_parameter("b", [D, 1], F32, isOutput=False)
    cvA = nc.declare_dram_parameter("cvA", [L, D], F32, isOutput=True)
    cvB = nc.declare_dram_parameter("cvB", [L, D], F32, isOutput=True)

    TMIN_d = nc.dram_tensor("TMIN_d", [L, L], F32)  # fully-masked scores

    def r(ap):
        return ap.bitcast(F32R)

    with tile.TileContext(nc) as tc, ExitStack() as ctx:
        glob = ctx.enter_context(tc.tile_pool(name="glob", bufs=1))

        # identities first: the first PE transposes depend only on these
        ident_bf = glob.tile([128, 128], BF16)
        make_identity(nc, ident_bf)
        ident_f = glob.tile([128, 128], F32)
        make_identity(nc, ident_f)
        ident_r = glob.tile([128, 128], F32R)
        nc.vector.tensor_copy(out=ident_r, in_=ident_f)

        ones_bf = glob.tile([128, 1], BF16)
        nc.vector.memset(ones_bf, 1.0)
        ones_f = glob.tile([1, 128], F32)
        nc.vector.memset(ones_f, 1.0)
        ones_r = glob.tile([1, 128], F32R)  # rank-1 bias lhsT
        nc.vector.tensor_copy(out=ones_r, in_=ones_f)

        b_t = glob.tile([128, KC], F32)
        b_tr = glob.tile([128, KC], F32R)
        maA = glob.tile([128, NS], F32)
        maA_min = glob.tile([128, NS], F32)  # 1 -> +BIG, 0 -> NEG
        MBb = glob.tile([128, L], BF16)  # maskB min-mask
        bv = glob.tile([1, L], F32R)  # b @ B^T (rank-1 score bias)
        cmax_full = glob.tile([128, L], F32)  # running colmax of masked scores
        nc.gpsimd.memset(cmax_full, -3.0e38)
        a_bf = glob.tile([128, NS, D], BF16)  # [l-part, lc, e] for phase C
        b_bf = glob.tile([128, NS, D], BF16)  # [m-part, mc, e] for cvB

        # K-pool spans preamble + pass 1; freed before phase C
        with tc.tile_pool(name="kpool", bufs=1) as kpl:
            K = kpl.tile([128, KC, L], F32R)  # K[p,dc,m] = (W @ B^T)[dc*128+p, m]

            # small-globals scope (temp f32 maskB row freed before main preamble)
            with tc.tile_pool(name="pre0", bufs=1) as pre0:
                # small-input DMAs go on the ACT hwdge queue so the W/B strip
                # loads on the SP queue are not stuck behind them
                nc.scalar.dma_start(
                    out=b_t, in_=bp[:].rearrange("(c p) o -> p (c o)", p=128)
                )
                nc.scalar.dma_start(
                    out=maA, in_=maskA[:].rearrange("(s p) o -> p (s o)", p=128)
                )
                nc.vector.tensor_scalar(
                    out=maA_min, in0=maA, scalar1=BIG - NEG, scalar2=NEG,
                    op0=MULT, op1=ADD,
                )
                nc.vector.tensor_copy(out=b_tr, in_=b_t)
                MBf = pre0.tile([128, L], F32)
                nc.scalar.dma_start(
                    out=MBf,
                    in_=maskB[:].rearrange("o n -> (o n)").partition_broadcast(128),
                )
                nc.vector.tensor_scalar(
                    out=MBb, in0=MBf, scalar1=BIG - NEG, scalar2=NEG,
                    op0=MULT, op1=ADD,
                )

            # ---------------- Preamble: WT, K = W @ B^T, bv, b_bf ----------------
            with tc.tile_pool(name="pre", bufs=1) as pre, \
                 tc.tile_pool(name="pre_ps", bufs=1, space="PSUM") as pre_ps:
                WT = pre.tile([128, KC, D], F32R)  # WT[p,ec,d] = W[d, ec*128+p]
                for wc in range(KC):
                    stripW = pre.tile([128, D], F32R, tag="strip", bufs=2)
                    nc.scalar.dma_start(
                        out=stripW, in_=r(Wp[wc * 128:(wc + 1) * 128, :])
                    )
                    for g in range(2):
                        tpw = pre_ps.tile([128, 4, 128], F32R, tag="tp", bufs=2)
                        for j in range(4):
                            ec = g * 4 + j
                            nc.tensor.transpose(
                                tpw[:, j, :],
                                stripW[:, ec * 128:(ec + 1) * 128],
                                ident_r,
                            )
                        nc.scalar.copy(
                            out=WT[:, g * 4:(g + 1) * 4, wc * 128:(wc + 1) * 128],
                            in_=tpw,
                        )

                for p in range(8):  # m-pairs of 256
                    btcol = pre.tile([128, KC, 256], F32R, tag="btcol", bufs=2)
                    for s in range(2):
                        mc = p * 2 + s
                        stripB = pre.tile([128, D], F32R, tag="strip", bufs=2)
                        nc.sync.dma_start(
                            out=stripB, in_=r(inputB[mc * 128:(mc + 1) * 128, :])
                        )
                        nc.gpsimd.tensor_copy(out=b_bf[:, mc, :], in_=stripB)
                        for g in range(2):
                            tpb = pre_ps.tile([128, 4, 128], F32R, tag="tp", bufs=2)
                            for j in range(4):
                                ec = g * 4 + j
                                nc.tensor.transpose(
                                    tpb[:, j, :],
                                    stripB[:, ec * 128:(ec + 1) * 128],
                                    ident_r,
                                )
                            nc.vector.tensor_copy(
                                out=btcol[:, g * 4:(g + 1) * 4,
                                          s * 128:(s + 1) * 128],
                                in_=tpb,
                            )
                    # K[:, :, p-slice] accumulation (f32r, ap=256)
                    for h in range(4):  # dc pairs
                        kps = pre_ps.tile([128, 2, 256], F32, tag="kps", bufs=2)
                        for dd in range(2):
                            dc = h * 2 + dd
                            for ec in range(KC):
                                nc.tensor.matmul(
                                    kps[:, dd, :],
                                    WT[:, ec, dc * 128:(dc + 1) * 128],
                                    btcol[:, ec, :],
                                    start=(ec == 0),
                                    stop=(ec == KC - 1),
                                )
                        nc.scalar.copy(
                            out=K[:, h * 2:(h + 1) * 2, p * 256:(p + 1) * 256],
                            in_=kps,
                        )
                    # bias row bv[p-slice] = b @ B^T (after K: off the critical path)
                    bvp = pre_ps.tile([1, 256], F32, tag="bvp", bufs=1)
                    for ec in range(KC):
                        nc.tensor.matmul(
                            bvp,
                            b_tr[:, ec:ec + 1],
                            btcol[:, ec, :],
                            start=(ec == 0),
                            stop=(ec == KC - 1),
                        )
                    nc.scalar.copy(out=bv[0:1, p * 256:(p + 1) * 256], in_=bvp)

            # ---------------- Pass 1: S strips, E_B, cvB, colmax ----------------
            with tc.tile_pool(name="p1", bufs=1) as p1, \
                 tc.tile_pool(name="p1_ps", bufs=1, space="PSUM") as p1_ps:
                smasks = {}
                ebs = {}
                astrips = {}

                def a_load(i):
                    t = p1.tile([128, D], F32R, tag="stripA", bufs=2)
                    nc.sync.dma_start(
                        out=t, in_=r(inputA[i * 128:(i + 1) * 128, :])
                    )
                    astrips[i] = t

                a_load(0)

                ats = {}

                def stage_s_pre(i):
                    # A strip (prefetched): PE transpose + at copy (ACT first)
                    stripA = astrips.pop(i)
                    if i + 1 < NS:
                        a_load(i + 1)
                    tpa = p1_ps.tile([128, KC, 128], F32R, tag="tpa", bufs=1)
                    for dc in range(KC):
                        nc.tensor.transpose(
                            tpa[:, dc, :],
                            stripA[:, dc * 128:(dc + 1) * 128],
                            ident_r,
                        )
                    at = p1.tile([128, KC, 128], F32R, tag="at", bufs=2)
                    nc.scalar.copy(out=at, in_=tpa)
                    nc.gpsimd.tensor_copy(out=a_bf[:, i, :], in_=stripA)
                    ats[i] = at

                def stage_s_mm(i):
                    # S matmuls + maskB min
                    at = ats.pop(i)
                    smask = p1.tile([128, L], F32, tag="smask", bufs=2)
                    for q in range(4):
                        sps = p1_ps.tile([128, 512], F32, tag="ps2k", bufs=4)
                        msl = slice(q * 512, (q + 1) * 512)
                        nc.tensor.matmul(
                            sps, ones_r, bv[0:1, msl], start=True, stop=False,
                        )
                        for dc in range(KC):
                            nc.tensor.matmul(
                                sps,
                                at[:, dc, :],
                                K[:, dc, msl],
                                start=False,
                                stop=(dc == KC - 1),
                            )
                        nc.vector.tensor_tensor(
                            out=smask[:, msl], in0=sps, in1=MBb[:, msl], op=MIN
                        )
                    smasks[i] = smask

                def stage_t(i):
                    # row softmax stats, E_B, TMIN scratch, running colmax
                    smask = smasks.pop(i)
                    negrm = p1.tile([128, 1], F32, tag="negrm", bufs=2)
                    nc.vector.reduce_max(out=negrm, in_=smask, axis=X, negate=True)
                    biasB = p1.tile([128, 1], F32, tag="biasB", bufs=2)
                    nc.vector.tensor_tensor(
                        out=biasB, in0=negrm, in1=maA[:, i:i + 1], op=MULT
                    )
                    eb = p1.tile([128, L], BF16, tag="eb", bufs=2)
                    denomB = p1.tile([128, 1], F32, tag="denomB", bufs=2)
                    nc.scalar.activation(
                        out=eb, in_=smask, func=EXP,
                        bias=biasB, scale=maA[:, i:i + 1], accum_out=denomB,
                    )
                    # fully-masked scores (A-mask applied too) -> DRAM for phase C
                    if i == NS - 1:
                        nc.vector.tensor_scalar_min(smask, smask, maA_min[:, i:i + 1])
                    else:
                        nc.gpsimd.tensor_scalar_min(smask, smask, maA_min[:, i:i + 1])
                    nc.sync.dma_start(out=TMIN_d[i * 128:(i + 1) * 128, :], in_=smask)
                    # per-strip column max -> running colmax (gpsimd engine)
                    for h in range(2):
                        ar = p1.tile([128, 1024], F32, tag="ar", bufs=1)
                        hsl = slice(h * 1024, (h + 1) * 1024)
                        nc.gpsimd.partition_all_reduce(
                            ar, smask[:, hsl], channels=128,
                            reduce_op=bass_isa.ReduceOp.max,
                        )
                        nc.vector.tensor_tensor(
                            out=cmax_full[:, hsl], in0=cmax_full[:, hsl],
                            in1=ar, op=MAXOP,
                        )
                    ebs[i] = (eb, denomB)

                def stage_v(i):
                    # cvB strip
                    eb, denomB = ebs.pop(i)
                    ebt = p1.tile([128, NS, 128], BF16, tag="ebt", bufs=2)
                    for g in range(2):
                        tp3 = p1_ps.tile([128, 8, 128], BF16, tag="ps2k", bufs=4)
                        for j in range(8):
                            mc = g * 8 + j
                            nc.tensor.transpose(
                                tp3[:, j, :], eb[:, mc * 128:(mc + 1) * 128],
                                ident_bf,
                            )
                        nc.scalar.copy(out=ebt[:, g * 8:(g + 1) * 8, :], in_=tp3)
                    ups = p1_ps.tile([128, D], F32, tag="ups", bufs=1)
                    for nb in range(2):
                        for mc in range(NS):
                            nc.tensor.matmul(
                                ups[:, nb * 512:(nb + 1) * 512],
                                ebt[:, mc, :],
                                b_bf[:, mc, nb * 512:(nb + 1) * 512],
                                start=(mc == 0),
                                stop=(mc == NS - 1),
                            )
                    rden = p1.tile([128, 1], F32, tag="rden", bufs=2)
                    nc.vector.reciprocal(out=rden, in_=denomB)
                    cvb_sb = p1.tile([128, D], F32, tag="cvb_sb", bufs=1)
                    nc.vector.tensor_scalar(
                        out=cvb_sb, in0=ups, scalar1=rden, scalar2=None, op0=MULT
                    )
                    nc.sync.dma_start(out=cvB[i * 128:(i + 1) * 128, :], in_=cvb_sb)

                for i in range(NS):
                    stage_s_pre(i)
                    if i >= 2:
                        stage_v(i - 2)
                    stage_s_mm(i)
                    if i >= 1:
                        stage_t(i - 1)
                stage_t(NS - 1)
                stage_v(NS - 2)
                stage_v(NS - 1)

        # ---------------- Phase C: cvA per m-chunk ----------------
        with tc.tile_pool(name="pc", bufs=1) as pc, \
             tc.tile_pool(name="pc_ps", bufs=1, space="PSUM") as pc_ps:
            cprev = {}
            subs_last = {}

            def c_stage1(j):
                # tmin column slab: [l-part, lc, m] for 128 columns m
                slab = pc.tile([128, NS, 128], F32, tag="slab", bufs=6)
                if j < 3:
                    nc.sync.dma_start(
                        out=slab[:, 0:NS - 1, :],
                        in_=TMIN_d[0:(NS - 1) * 128, j * 128:(j + 1) * 128]
                        .rearrange("(c p) m -> p c m", p=128),
                    )
                    nc.sync.dma_start(
                        out=slab[:, NS - 1, :],
                        in_=TMIN_d[(NS - 1) * 128:, j * 128:(j + 1) * 128]
                        .rearrange("(c p) m -> p (c m)", p=128),
                    )
                else:
                    nc.sync.dma_start(
                        out=slab,
                        in_=TMIN_d[:, j * 128:(j + 1) * 128].rearrange(
                            "(c p) m -> p c m", p=128
                        ),
                    )
                # E_A tiles = exp(tmin - colmax), bf16 (colmax from pass 1)
                cmb = cmax_full[:, j * 128:(j + 1) * 128].unsqueeze(1).broadcast_to(
                    (128, 4, 128)
                )
                eng = nc.vector if j < 3 else nc.gpsimd
                for q in range(4):
                    sub_inst = eng.tensor_tensor(
                        out=slab[:, q * 4:(q + 1) * 4, :],
                        in0=slab[:, q * 4:(q + 1) * 4, :], in1=cmb, op=SUB,
                    )
                subs_last[j] = sub_inst
                ea_t = pc.tile([128, NS, 128], BF16, tag="ea_t", bufs=6)
                for hh in range(4):
                    nc.scalar.activation(
                        out=ea_t[:, hh * 4:(hh + 1) * 4, :],
                        in_=slab[:, hh * 4:(hh + 1) * 4, :], func=EXP,
                    )
                cprev[j] = ea_t

            def c_stage2(j):
                ea_t = cprev.pop(j)
                aps = pc_ps.tile([128, D], F32, tag="aps", bufs=3)
                csum = pc_ps.tile([128, 1], F32, tag="csum", bufs=2)
                for lc in range(NS):
                    nc.tensor.matmul(
                        csum, ea_t[:, lc, :], ones_bf,
                        start=(lc == 0), stop=(lc == NS - 1),
                    )
                for nb in range(2):
                    for lc in range(NS):
                        nc.tensor.matmul(
                            aps[:, nb * 512:(nb + 1) * 512],
                            ea_t[:, lc, :],
                            a_bf[:, lc, nb * 512:(nb + 1) * 512],
                            start=(lc == 0),
                            stop=(lc == NS - 1),
                        )
                rcs = pc.tile([128, 1], F32, tag="rcs", bufs=2)
                nc.vector.reciprocal(out=rcs, in_=csum)
                cva_sb = pc.tile([128, D], F32, tag="cva_sb", bufs=2)
                nc.vector.tensor_scalar(
                    out=cva_sb, in0=aps, scalar1=rcs, scalar2=None, op0=MULT
                )
                nc.sync.dma_start(out=cvA[j * 128:(j + 1) * 128, :], in_=cva_sb)

            c_stage1(0)
            c_stage1(1)
            c_stage1(2)
            c_stage1(3)
            for j in range(NS):
                if j + 4 < NS:
                    c_stage1(j + 4)
                c_stage2(j)
    if not nc.is_finalized():
        nc.finalize()
    return nc


def run(inputs, trace=False, trace_kwargs=None):
    if "nc" not in _CACHE:
        _CACHE["nc"] = build()
    nc = _CACHE["nc"]
    in_maps = []
    for i in range(B):
        in_maps.append({
            "inputA": np.ascontiguousarray(inputs["inputA"][i], dtype=np.float32),
            "inputB": np.ascontiguousarray(inputs["inputB"][i], dtype=np.float32),
            "maskA": np.ascontiguousarray(
                inputs["maskA"][i], dtype=np.float32).reshape(L, 1),
            "maskB": np.ascontiguousarray(
                inputs["maskB"][i], dtype=np.float32).reshape(1, L),
            "W": np.ascontiguousarray(inputs["W"], dtype=np.float32),
            "b": np.ascontiguousarray(inputs["b"], dtype=np.float32).reshape(D, 1),
        })
    try:
        res = run_bass_kernel_spmd(
            nc, in_maps, core_ids=list(range(B)), trace=trace,
            **(trace_kwargs or {}),
        )
    except ModuleNotFoundError:
        res = run_bass_kernel_spmd(nc, in_maps, core_ids=list(range(B)), trace=False)
    cva = np.stack([res.results[i]["cvA"] for i in range(B)]).astype(np.float32)
    cvb = np.stack([res.results[i]["cvB"] for i in range(B)]).astype(np.float32)
    return (cva, cvb), res


def kernel(**inputs):
    (cva, cvb), _ = run(inputs, trace=False)
    return cva, cvb
